# revision 4
# baseline (speedup 1.0000x reference)
"""Causal MHA + RoPE on 8 TRN2 NeuronCores — v5.1.

Sharding: 8 cores = 2 batch x 4 head-groups; each core does 4 heads of one
batch (QKV proj sliced to EL=256 out dims, full attention for those heads,
partial O-proj summed on host).

Structure per core:
- Q/K projections produce transposed [dk, seq] layouts with RoPE applied via
  sin-premultiply + SBUF->SBUF DMA 32-block swap (sign folded into host sin).
- Flash-style causal attention per head-pair: scores (K=64 pairs run
  concurrently via tile_position), exp on scalar engine (2-segment AP on
  diagonal tiles), wedge-band causal mask on DVE, AV accumulation with an
  appended ones-row for the softmax denominator.
- Attention inner loop software-pipelined: scores(t+1) issued to the PE
  before AV(t) so the PE never idles behind the scalar exp.
- Both head-pair phases of one q-span interleaved; output projection per
  span follows immediately; y written bf16.
"""
import math
import numpy as np
import ml_dtypes

import concourse.bass as bass
import concourse.mybir as mybir
import concourse.tile as tile
from concourse import bacc
from concourse.bass import ds
from concourse.bass_utils import run_bass_kernel_spmd

F32 = mybir.dt.float32
BF16 = mybir.dt.bfloat16
EXP = mybir.ActivationFunctionType.Exp

D_MODEL = 1024
DK = 64
THETA = 10000.0
B, S = 2, 2048
HPC = 4
EL = HPC * DK
SCALE = 1.0 / math.sqrt(DK)
NQ = 512
NT = 128
NKT = S // NT
DCH = D_MODEL // 128
VW = DK + 1

_CACHE = {}


def _build_nc():
    nc = bacc.Bacc(None, target_bir_lowering=False)
    xT = nc.declare_dram_parameter("xT", [D_MODEL, S], BF16, isOutput=False)
    wq = nc.declare_dram_parameter("wq", [D_MODEL, EL], BF16, isOutput=False)
    wk = nc.declare_dram_parameter("wk", [D_MODEL, EL], BF16, isOutput=False)
    wv = nc.declare_dram_parameter("wv", [D_MODEL, EL], BF16, isOutput=False)
    wo = nc.declare_dram_parameter("wo", [EL, D_MODEL], BF16, isOutput=False)
    cosT = nc.declare_dram_parameter("cosT", [128, S], BF16, isOutput=False)
    sinT = nc.declare_dram_parameter("sinT", [128, S], BF16, isOutput=False)
    y = nc.declare_dram_parameter("y", [S, D_MODEL], BF16, isOutput=True)

    with tile.TileContext(nc) as tc:
        with (
            tc.tile_pool(name="p_fin", bufs=1) as p_fin,
            tc.tile_pool(name="p_work", bufs=1) as p_work,
            tc.tile_pool(name="ps", bufs=1, space="PSUM") as ps,
        ):
            # ---- persistent tiles ----
            qt_fin = p_fin.tile([128, 2 * S], BF16, tag="qt_fin", name="qt_fin")
            kt_fin = p_fin.tile([128, 2 * S], BF16, tag="kt_fin", name="kt_fin")
            v_aug = p_fin.tile([128, NKT * HPC * VW], BF16, tag="v_aug", name="v_aug")
            attnT = [p_fin.tile([128, S], BF16, tag=f"attnT{p}", name=f"attnT{p}")
                     for p in range(2)]

            # ---- PE warmup: junk matmuls with no DMA deps ----
            junk = p_fin.tile([128, NQ], BF16, tag="junk", name="junk")
            nc.vector.memset(junk, 0.125)
            for _ in range(8):
                wps = ps.tile([128, NQ], F32, tag="acc", bufs=4, name="wps")
                nc.tensor.matmul(wps[:, 0:NQ], junk[:, 0:128], junk[:, 0:NQ],
                                 start=True, stop=True)

            # ---- input DMAs, ordered by first use, spread over 3 queues ----
            xp = {}
            for j2 in range(2):
                xp[j2] = [p_fin.tile([128, 1024], BF16, tag="xt", bufs=16,
                                     name=f"x{j2}_{d}") for d in range(DCH)]
            wq_sb = p_fin.tile([128, DCH * EL], BF16, tag="wq", name="wq_sb")
            wk_sb = p_fin.tile([128, DCH * EL], BF16, tag="wk", name="wk_sb")
            wv_sb = p_fin.tile([128, DCH * EL], BF16, tag="wv", name="wv_sb")

            def dma_x(j2, d, eng):
                eng.dma_start(out=xp[j2][d],
                              in_=xT[d * 128:(d + 1) * 128,
                                     j2 * 1024:(j2 + 1) * 1024])

            # first-use order: wq_c0 | x d0.. , wk_c0, wv interleaved
            for c in range(2):
                nc.scalar.dma_start(
                    out=wq_sb.rearrange("p (d c f) -> p d c f", d=DCH, c=2)[:, :, c],
                    in_=wq.rearrange("(d p) (c f) -> p d c f", p=128, c=2)[:, :, c])
                nc.gpsimd.dma_start(
                    out=wk_sb.rearrange("p (d c f) -> p d c f", d=DCH, c=2)[:, :, c],
                    in_=wk.rearrange("(d p) (c f) -> p d c f", p=128, c=2)[:, :, c])
            for d in range(3):
                dma_x(0, d, nc.sync)
            for d in range(3, 6):
                dma_x(0, d, nc.scalar)
            for d in range(6, 8):
                dma_x(0, d, nc.gpsimd)
            nc.gpsimd.dma_start(out=wv_sb.rearrange("p (d e) -> p d e", d=DCH),
                                in_=wv.rearrange("(d p) e -> p d e", p=128))
            cos_sb = p_fin.tile([128, S], BF16, tag="cos", name="cos_sb")
            sin_sb = p_fin.tile([128, S], BF16, tag="sin", name="sin_sb")
            for j2 in range(2):
                sl = ds(j2 * 1024, 1024)
                nc.scalar.dma_start(out=cos_sb[:, sl], in_=cosT[:, sl])
                nc.scalar.dma_start(out=sin_sb[:, sl], in_=sinT[:, sl])
            wo_sb = p_fin.tile([128, 2 * D_MODEL], BF16, tag="wo", name="wo_sb")
            nc.sync.dma_start(out=wo_sb.rearrange("p (c e) -> p c e", c=2),
                              in_=wo.rearrange("(c p) e -> p c e", p=128))

            # v_aug: set everything to 1.0 once; V copies overwrite [0:DK].
            nc.gpsimd.memset(v_aug, 1.0)

            # shared causal triangle for the diagonal wedge band:
            # tri[k, h, b] = 1.0 if b >= k else 0.0
            tri = p_fin.tile([128, 2 * NT], BF16, tag="tri", name="tri")
            nc.vector.memset(tri, 1.0)
            nc.gpsimd.affine_select(
                out=tri.rearrange("p (h b) -> p h b", h=2),
                in_=tri.rearrange("p (h b) -> p h b", h=2),
                compare_op=mybir.AluOpType.is_ge,
                fill=0.0, base=0,
                pattern=[[0, 2], [1, NT]],
                channel_multiplier=-1,
            )

            # ---- phase emitters ----
            def emit_b(j2):
                if j2 == 1:
                    for d in range(4):
                        dma_x(1, d, nc.sync)
                    for d in range(4, 8):
                        dma_x(1, d, nc.gpsimd)
                sl2 = ds(j2 * 1024, 1024)
                for kind, w_sb in ((0, wq_sb), (1, wk_sb)):
                    fin = qt_fin if kind == 0 else kt_fin
                    u2 = p_work.tile([128, 2048], BF16, tag="u", bufs=2,
                                     name="u2")
                    us2 = p_work.tile([128, 2048], BF16, tag="us", bufs=2,
                                      name="us2")
                    for c in range(2):
                        pq = ps.tile([128, 1024], F32, tag="big", bufs=2, name="pq")
                        for h5 in range(2):
                            for d in range(DCH):
                                nc.tensor.matmul(
                                    pq[:, ds(h5 * NQ, NQ)],
                                    w_sb[:, ds(d * EL + c * 128, 128)],
                                    xp[j2][d][:, ds(h5 * NQ, NQ)],
                                    start=(d == 0), stop=(d == DCH - 1))
                        raw = p_work.tile([128, 1024], BF16, tag="raw", bufs=4,
                                          name="raw")
                        nc.vector.tensor_copy(raw, pq)
                        nc.vector.tensor_mul(u2[:, ds(c * 1024, 1024)],
                                             raw, sin_sb[:, sl2])
                        nc.vector.tensor_mul(fin[:, ds(c * S + j2 * 1024, 1024)],
                                             raw, cos_sb[:, sl2])
                    for blk in range(4):
                        src = (blk ^ 1) * 32
                        nc.sync.dma_start(out=us2[blk * 32:(blk + 1) * 32, :],
                                          in_=u2[src:src + 32, :])
                    for c in range(2):
                        fsl = ds(c * S + j2 * 1024, 1024)
                        nc.vector.tensor_add(fin[:, fsl], fin[:, fsl],
                                             us2[:, ds(c * 1024, 1024)])
                for sti in range(8):
                    t = 8 * j2 + sti
                    pv = ps.tile([128, 1024], F32, tag="big", bufs=2, name="pv")
                    for d in range(DCH):
                        nc.tensor.matmul(
                            pv[:, 0:EL],
                            xp[j2][d][:, ds(sti * 128, 128)],
                            wv_sb[:, ds(d * EL, EL)],
                            start=(d == 0), stop=(d == DCH - 1))
                    vview = v_aug[:, ds(t * HPC * VW, HPC * VW)].rearrange(
                        "p (h a) -> p h a", a=VW)
                    nc.vector.tensor_copy(
                        vview[:, :, 0:DK],
                        pv[:, 0:EL].rearrange("p (h m) -> p h m", m=DK))

            def emit_attn2(j):
                """Both head-pair phases (p=0,1) of q-span j, interleaved and
                software-pipelined: scores(t+1) hits the PE queue before
                AV(t) so the PE isn't stalled behind the scalar exp."""
                ntile = 4 * j + 4
                pv_acc = {p: (ps.tile([128, NQ], F32, tag="acc", bufs=4,
                                      name=f"pva{p}"),
                              ps.tile([128, NQ], F32, tag="acc", bufs=4,
                                      name=f"pvb{p}"))
                          for p in range(2)}

                def emit_scores_exp(p, t):
                    dd = max(0, t - 4 * j)
                    q0 = dd * NT
                    w = NQ - q0
                    stp = ps.tile([128, 2 * NQ], F32, tag="big", bufs=2,
                                  name="stp")
                    for hh in range(2):
                        nc.tensor.matmul(
                            stp[:, ds(hh * NQ + q0, w)],
                            kt_fin[hh * 64:(hh + 1) * 64, ds(p * S + t * NT, NT)],
                            qt_fin[hh * 64:(hh + 1) * 64,
                                   ds(p * S + j * NQ + q0, w)],
                            start=True, stop=True)
                    ste = p_work.tile([128, 2 * NQ], BF16, tag="ste", bufs=8,
                                      name="ste")
                    if dd or t == 4 * j:
                        stp_v = stp.rearrange("p (h q) -> p h q", h=2)[:, :, q0:NQ]
                        ste_v = ste.rearrange("p (h q) -> p h q", h=2)[:, :, q0:NQ]
                        nc.scalar.activation(ste_v, stp_v, EXP, scale=SCALE)
                        band = ste.rearrange("p (h q) -> p h q", h=2)[:, :,
                                                                      q0:q0 + NT]
                        nc.vector.tensor_mul(
                            band, band, tri.rearrange("p (h b) -> p h b", h=2))
                    else:
                        nc.scalar.activation(ste, stp, EXP, scale=SCALE)
                    return ste

                def emit_av(p, t, ste):
                    dd = max(0, t - 4 * j)
                    q0 = dd * NT
                    w = NQ - q0
                    pva, pvb = pv_acc[p]
                    for hh, pvx in ((0, pva), (1, pvb)):
                        nc.tensor.matmul(
                            pvx[0:VW, ds(q0, w)],
                            v_aug[:, ds(t * HPC * VW + (2 * p + hh) * VW, VW)],
                            ste[:, ds(hh * NQ + q0, w)],
                            start=(t == 0), stop=(t == ntile - 1))

                # pipeline: units are (p, t) pairs in interleaved order
                units = [(p, t) for t in range(ntile) for p in range(2)]
                pending = []
                for u in units:
                    ste = emit_scores_exp(*u)
                    pending.append((u, ste))
                    if len(pending) > 2:
                        (pu, pste) = pending.pop(0)
                        emit_av(*pu, pste)
                for (pu, pste) in pending:
                    emit_av(*pu, pste)

                for p in range(2):
                    pva, pvb = pv_acc[p]
                    lcp_a = p_work.tile([1, NQ], F32, tag="lcp_a", bufs=3, name="lcp_a")
                    lcp_b = p_work.tile([1, NQ], F32, tag="lcp_b", bufs=3, name="lcp_b")
                    nc.vector.tensor_copy(lcp_a, pva[64:65, :])
                    nc.vector.tensor_copy(lcp_b, pvb[64:65, :])
                    recl_a = p_work.tile([1, NQ], F32, tag="recl_a", bufs=3, name="recl_a")
                    recl_b = p_work.tile([1, NQ], F32, tag="recl_b", bufs=3, name="recl_b")
                    nc.vector.reciprocal_approx_fast(recl_a, lcp_a)
                    nc.vector.reciprocal_approx_fast(recl_b, lcp_b)
                    rb_a = p_work.tile([64, NQ], F32, tag="rb_a", bufs=3, name="rb_a")
                    rb_b = p_work.tile([64, NQ], F32, tag="rb_b", bufs=3, name="rb_b")
                    nc.gpsimd.partition_broadcast(rb_a, recl_a, channels=64)
                    nc.gpsimd.partition_broadcast(rb_b, recl_b, channels=64)
                    sl = ds(j * NQ, NQ)
                    nc.vector.tensor_mul(attnT[p][0:64, sl], pva[0:64, :], rb_a)
                    nc.vector.tensor_mul(attnT[p][64:128, sl], pvb[0:64, :], rb_b)

            def emit_e(j):
                for sti in range(4 * j, 4 * j + 4):
                    ysb = p_work.tile([128, 1024], BF16, tag="ysb", bufs=3,
                                      name="ysb")
                    for e2 in range(2):
                        py = ps.tile([128, NQ], F32, tag="acc", bufs=4, name="py")
                        for c in range(2):
                            nc.tensor.matmul(
                                py[:, 0:NQ],
                                attnT[c][:, ds(sti * 128, 128)],
                                wo_sb[:, ds(c * D_MODEL + e2 * NQ, NQ)],
                                start=(c == 0), stop=(c == 1))
                        nc.vector.tensor_copy(ysb[:, ds(e2 * NQ, NQ)], py[:, 0:NQ])
                    eng = nc.sync if sti % 2 == 0 else nc.gpsimd
                    eng.dma_start(
                        out=y[sti * 128:(sti + 1) * 128, :],
                        in_=ysb)

            emit_b(0)
            emit_attn2(0)
            emit_b(1)
            emit_attn2(1)
            emit_e(0)
            emit_attn2(2)
            emit_e(1)
            emit_attn2(3)
            emit_e(2)
            emit_e(3)
    nc.finalize()
    return nc


def _host_prep(x, Wq, Wk, Wv, Wo):
    x = np.asarray(x, dtype=np.float32)
    Wq, Wk, Wv, Wo = (np.asarray(w, dtype=np.float32) for w in (Wq, Wk, Wv, Wo))
    bf = ml_dtypes.bfloat16

    p64 = np.concatenate([np.arange(0, DK, 2), np.arange(1, DK, 2)])
    freqs = 1.0 / THETA ** (np.arange(0, DK, 2, dtype=np.float64) / DK)
    ang = np.arange(S, dtype=np.float64)[None, :] * freqs[:, None]
    cos32 = np.cos(ang).astype(np.float32)
    sin32 = np.sin(ang).astype(np.float32)
    cosT = np.ascontiguousarray(np.tile(cos32, (4, 1))).astype(bf)
    # sin_alt: pre-swap layout [s, -s, s, -s]; the kernel multiplies FIRST,
    # then swaps 32-blocks, landing [-s, s, -s, s] contributions.
    sinT = np.ascontiguousarray(
        np.concatenate([sin32, -sin32, sin32, -sin32], axis=0)).astype(bf)

    xTs = [np.ascontiguousarray(x[b].T).astype(bf) for b in range(B)]
    perm = np.concatenate([h * DK + p64 for h in range(HPC)])

    in_maps = []
    for core in range(8):
        bg, hg = core // 4, core % 4
        sl = slice(hg * EL, (hg + 1) * EL)
        in_maps.append({
            "xT": xTs[bg],
            "wq": np.ascontiguousarray(Wq[sl][perm].T).astype(bf),
            "wk": np.ascontiguousarray(Wk[sl][perm].T).astype(bf),
            "wv": np.ascontiguousarray(Wv[sl].T).astype(bf),
            "wo": np.ascontiguousarray(Wo[:, sl].T).astype(bf),
            "cosT": cosT,
            "sinT": sinT,
        })
    return in_maps


def kernel(x, Wq, Wk, Wv, Wo, _trace=False):
    if "nc" not in _CACHE:
        _CACHE["nc"] = _build_nc()
    nc = _CACHE["nc"]
    in_maps = _host_prep(x, Wq, Wk, Wv, Wo)
    res = run_bass_kernel_spmd(nc, in_maps, core_ids=list(range(8)), trace=_trace)
    _CACHE["last_result"] = res
    out = np.zeros((B, S, D_MODEL), dtype=np.float32)
    for core in range(8):
        out[core // 4] += np.asarray(res.results[core]["y"], dtype=np.float32)
    return out


# revision 7
# speedup vs baseline: 1.2248x; 1.2248x over previous
"""Causal MHA + RoPE on 8 TRN2 NeuronCores — v5.1.

Sharding: 8 cores = 2 batch x 4 head-groups; each core does 4 heads of one
batch (QKV proj sliced to EL=256 out dims, full attention for those heads,
partial O-proj summed on host).

Structure per core:
- Q/K projections produce transposed [dk, seq] layouts with RoPE applied via
  sin-premultiply + SBUF->SBUF DMA 32-block swap (sign folded into host sin).
- Flash-style causal attention per head-pair: scores (K=64 pairs run
  concurrently via tile_position), exp on scalar engine (2-segment AP on
  diagonal tiles), wedge-band causal mask on DVE, AV accumulation with an
  appended ones-row for the softmax denominator.
- Attention inner loop software-pipelined: scores(t+1) issued to the PE
  before AV(t) so the PE never idles behind the scalar exp.
- Both head-pair phases of one q-span interleaved; output projection per
  span follows immediately; y written bf16.
"""
import math
import numpy as np
import ml_dtypes

import concourse.bass as bass
import concourse.mybir as mybir
import concourse.tile as tile
from concourse import bacc
from concourse.bass import ds
from concourse.bass_utils import run_bass_kernel_spmd

F32 = mybir.dt.float32
BF16 = mybir.dt.bfloat16
EXP = mybir.ActivationFunctionType.Exp

D_MODEL = 1024
DK = 64
THETA = 10000.0
B, S = 2, 2048
HPC = 4
EL = HPC * DK
SCALE = 1.0 / math.sqrt(DK)
NQ = 512
NT = 128
NKT = S // NT
DCH = D_MODEL // 128
VW = DK + 1

_CACHE = {}


def _build_nc():
    nc = bacc.Bacc(None, target_bir_lowering=False)
    xT = nc.declare_dram_parameter("xT", [D_MODEL, S], BF16, isOutput=False)
    wq = nc.declare_dram_parameter("wq", [D_MODEL, EL], BF16, isOutput=False)
    wk = nc.declare_dram_parameter("wk", [D_MODEL, EL], BF16, isOutput=False)
    wv = nc.declare_dram_parameter("wv", [D_MODEL, EL], BF16, isOutput=False)
    wo = nc.declare_dram_parameter("wo", [EL, D_MODEL], BF16, isOutput=False)
    cosT = nc.declare_dram_parameter("cosT", [128, S], BF16, isOutput=False)
    sinT = nc.declare_dram_parameter("sinT", [128, S], BF16, isOutput=False)
    y = nc.declare_dram_parameter("y", [S, D_MODEL], BF16, isOutput=True)

    with tile.TileContext(nc) as tc:
        with (
            tc.tile_pool(name="p_fin", bufs=1) as p_fin,
            tc.tile_pool(name="p_work", bufs=1) as p_work,
            tc.tile_pool(name="ps", bufs=1, space="PSUM") as ps,
        ):
            # ---- persistent tiles ----
            qt_fin = p_fin.tile([128, 2 * S], BF16, tag="qt_fin", name="qt_fin")
            kt_fin = p_fin.tile([128, 2 * S], BF16, tag="kt_fin", name="kt_fin")
            v_aug = p_fin.tile([128, NKT * HPC * VW], BF16, tag="v_aug", name="v_aug")
            attnT = [p_fin.tile([128, S], BF16, tag=f"attnT{p}", name=f"attnT{p}")
                     for p in range(2)]

            # ---- PE warmup: junk matmuls with no DMA deps ----
            junk = p_fin.tile([128, NQ], BF16, tag="junk", name="junk")
            nc.vector.memset(junk, 0.125)
            for _ in range(8):
                wps = ps.tile([128, NQ], F32, tag="acc", bufs=4, name="wps")
                nc.tensor.matmul(wps[:, 0:NQ], junk[:, 0:128], junk[:, 0:NQ],
                                 start=True, stop=True)

            # ---- input DMAs, ordered by first use, spread over 3 queues ----
            xp = {}
            for j2 in range(2):
                xp[j2] = [p_fin.tile([128, 1024], BF16, tag="xt", bufs=16,
                                     name=f"x{j2}_{d}") for d in range(DCH)]
            wq_sb = p_fin.tile([128, DCH * EL], BF16, tag="wq", name="wq_sb")
            wk_sb = p_fin.tile([128, DCH * EL], BF16, tag="wk", name="wk_sb")
            wv_sb = p_fin.tile([128, DCH * EL], BF16, tag="wv", name="wv_sb")

            def dma_x(j2, d, eng):
                eng.dma_start(out=xp[j2][d],
                              in_=xT[d * 128:(d + 1) * 128,
                                     j2 * 1024:(j2 + 1) * 1024])

            # first-use order: wq_c0 | x d0.. , wk_c0, wv interleaved
            for c in range(2):
                nc.scalar.dma_start(
                    out=wq_sb.rearrange("p (d c f) -> p d c f", d=DCH, c=2)[:, :, c],
                    in_=wq.rearrange("(d p) (c f) -> p d c f", p=128, c=2)[:, :, c])
                nc.gpsimd.dma_start(
                    out=wk_sb.rearrange("p (d c f) -> p d c f", d=DCH, c=2)[:, :, c],
                    in_=wk.rearrange("(d p) (c f) -> p d c f", p=128, c=2)[:, :, c])
            for d in range(3):
                dma_x(0, d, nc.sync)
            for d in range(3, 6):
                dma_x(0, d, nc.scalar)
            for d in range(6, 8):
                dma_x(0, d, nc.gpsimd)
            nc.gpsimd.dma_start(out=wv_sb.rearrange("p (d e) -> p d e", d=DCH),
                                in_=wv.rearrange("(d p) e -> p d e", p=128))
            cos_sb = p_fin.tile([128, S], BF16, tag="cos", name="cos_sb")
            sin_sb = p_fin.tile([128, S], BF16, tag="sin", name="sin_sb")
            for j2 in range(2):
                sl = ds(j2 * 1024, 1024)
                nc.scalar.dma_start(out=cos_sb[:, sl], in_=cosT[:, sl])
                nc.scalar.dma_start(out=sin_sb[:, sl], in_=sinT[:, sl])
            wo_sb = p_fin.tile([128, 2 * D_MODEL], BF16, tag="wo", name="wo_sb")
            nc.sync.dma_start(out=wo_sb.rearrange("p (c e) -> p c e", c=2),
                              in_=wo.rearrange("(c p) e -> p c e", p=128))

            # v_aug: set everything to 1.0 once; V copies overwrite [0:DK].
            nc.gpsimd.memset(v_aug, 1.0)

            # shared causal triangle for the diagonal wedge band:
            # tri[k, h, b] = 1.0 if b >= k else 0.0
            tri = p_fin.tile([128, 2 * NT], BF16, tag="tri", name="tri")
            nc.vector.memset(tri, 1.0)
            nc.gpsimd.affine_select(
                out=tri.rearrange("p (h b) -> p h b", h=2),
                in_=tri.rearrange("p (h b) -> p h b", h=2),
                compare_op=mybir.AluOpType.is_ge,
                fill=0.0, base=0,
                pattern=[[0, 2], [1, NT]],
                channel_multiplier=-1,
            )

            # ---- phase emitters ----
            def emit_b(j2):
                if j2 == 1:
                    for d in range(4):
                        dma_x(1, d, nc.sync)
                    for d in range(4, 8):
                        dma_x(1, d, nc.gpsimd)
                sl2 = ds(j2 * 1024, 1024)
                for kind, w_sb in ((0, wq_sb), (1, wk_sb)):
                    fin = qt_fin if kind == 0 else kt_fin
                    u2 = p_work.tile([128, 2048], BF16, tag="u", bufs=2,
                                     name="u2")
                    us2 = p_work.tile([128, 2048], BF16, tag="us", bufs=2,
                                      name="us2")
                    for c in range(2):
                        pq = ps.tile([128, 1024], F32, tag="big", bufs=2, name="pq")
                        for h5 in range(2):
                            for d in range(DCH):
                                nc.tensor.matmul(
                                    pq[:, ds(h5 * NQ, NQ)],
                                    w_sb[:, ds(d * EL + c * 128, 128)],
                                    xp[j2][d][:, ds(h5 * NQ, NQ)],
                                    start=(d == 0), stop=(d == DCH - 1))
                        raw = p_work.tile([128, 1024], BF16, tag="raw", bufs=4,
                                          name="raw")
                        nc.vector.tensor_copy(raw, pq)
                        nc.vector.tensor_mul(u2[:, ds(c * 1024, 1024)],
                                             raw, sin_sb[:, sl2])
                        nc.vector.tensor_mul(fin[:, ds(c * S + j2 * 1024, 1024)],
                                             raw, cos_sb[:, sl2])
                    for blk in range(4):
                        src = (blk ^ 1) * 32
                        nc.sync.dma_start(out=us2[blk * 32:(blk + 1) * 32, :],
                                          in_=u2[src:src + 32, :])
                    for c in range(2):
                        fsl = ds(c * S + j2 * 1024, 1024)
                        nc.vector.tensor_add(fin[:, fsl], fin[:, fsl],
                                             us2[:, ds(c * 1024, 1024)])
                for sti in range(8):
                    t = 8 * j2 + sti
                    pv = ps.tile([128, 1024], F32, tag="big", bufs=2, name="pv")
                    for d in range(DCH):
                        nc.tensor.matmul(
                            pv[:, 0:EL],
                            xp[j2][d][:, ds(sti * 128, 128)],
                            wv_sb[:, ds(d * EL, EL)],
                            start=(d == 0), stop=(d == DCH - 1))
                    vview = v_aug[:, ds(t * HPC * VW, HPC * VW)].rearrange(
                        "p (h a) -> p h a", a=VW)
                    nc.vector.tensor_copy(
                        vview[:, :, 0:DK],
                        pv[:, 0:EL].rearrange("p (h m) -> p h m", m=DK))

            def emit_attn(p, j):
                """One head-pair phase of q-span j, software-pipelined:
                scores(t+1) hits the PE queue before AV(t) so the PE isn't
                stalled behind the scalar exp."""
                ntile = 4 * j + 4
                pva = ps.tile([128, NQ], F32, tag="acc", bufs=4, name="pva")
                pvb = ps.tile([128, NQ], F32, tag="acc", bufs=4, name="pvb")

                def emit_scores_exp(t):
                    dd = max(0, t - 4 * j)
                    q0 = dd * NT
                    w = NQ - q0
                    stp = ps.tile([128, 2 * NQ], F32, tag="big", bufs=2,
                                  name="stp")
                    for hh in range(2):
                        nc.tensor.matmul(
                            stp[:, ds(hh * NQ + q0, w)],
                            kt_fin[hh * 64:(hh + 1) * 64, ds(p * S + t * NT, NT)],
                            qt_fin[hh * 64:(hh + 1) * 64,
                                   ds(p * S + j * NQ + q0, w)],
                            start=True, stop=True)
                    ste = p_work.tile([128, 2 * NQ], BF16, tag="ste", bufs=8,
                                      name="ste")
                    if dd or t == 4 * j:
                        stp_v = stp.rearrange("p (h q) -> p h q", h=2)[:, :, q0:NQ]
                        ste_v = ste.rearrange("p (h q) -> p h q", h=2)[:, :, q0:NQ]
                        nc.scalar.activation(ste_v, stp_v, EXP, scale=SCALE)
                        band = ste.rearrange("p (h q) -> p h q", h=2)[:, :,
                                                                      q0:q0 + NT]
                        nc.vector.tensor_mul(
                            band, band, tri.rearrange("p (h b) -> p h b", h=2))
                    else:
                        nc.scalar.activation(ste, stp, EXP, scale=SCALE)
                    return ste

                def emit_av(t, ste):
                    dd = max(0, t - 4 * j)
                    q0 = dd * NT
                    w = NQ - q0
                    for hh, pvx in ((0, pva), (1, pvb)):
                        nc.tensor.matmul(
                            pvx[0:VW, ds(q0, w)],
                            v_aug[:, ds(t * HPC * VW + (2 * p + hh) * VW, VW)],
                            ste[:, ds(hh * NQ + q0, w)],
                            start=(t == 0), stop=(t == ntile - 1))

                pending = []
                for t in range(ntile):
                    ste = emit_scores_exp(t)
                    pending.append((t, ste))
                    if len(pending) > 1:
                        (pt, pste) = pending.pop(0)
                        emit_av(pt, pste)
                for (pt, pste) in pending:
                    emit_av(pt, pste)

                lcp_a = p_work.tile([1, NQ], F32, tag="lcp_a", bufs=3, name="lcp_a")
                lcp_b = p_work.tile([1, NQ], F32, tag="lcp_b", bufs=3, name="lcp_b")
                nc.vector.tensor_copy(lcp_a, pva[64:65, :])
                nc.vector.tensor_copy(lcp_b, pvb[64:65, :])
                recl_a = p_work.tile([1, NQ], F32, tag="recl_a", bufs=3, name="recl_a")
                recl_b = p_work.tile([1, NQ], F32, tag="recl_b", bufs=3, name="recl_b")
                nc.vector.reciprocal_approx_fast(recl_a, lcp_a)
                nc.vector.reciprocal_approx_fast(recl_b, lcp_b)
                rb_a = p_work.tile([64, NQ], F32, tag="rb_a", bufs=3, name="rb_a")
                rb_b = p_work.tile([64, NQ], F32, tag="rb_b", bufs=3, name="rb_b")
                nc.gpsimd.partition_broadcast(rb_a, recl_a, channels=64)
                nc.gpsimd.partition_broadcast(rb_b, recl_b, channels=64)
                sl = ds(j * NQ, NQ)
                nc.vector.tensor_mul(attnT[p][0:64, sl], pva[0:64, :], rb_a)
                nc.vector.tensor_mul(attnT[p][64:128, sl], pvb[0:64, :], rb_b)

            def emit_e(j):
                for sti in range(4 * j, 4 * j + 4):
                    ysb = p_work.tile([128, 1024], BF16, tag="ysb", bufs=3,
                                      name="ysb")
                    for e2 in range(2):
                        py = ps.tile([128, NQ], F32, tag="acc", bufs=4, name="py")
                        for c in range(2):
                            nc.tensor.matmul(
                                py[:, 0:NQ],
                                attnT[c][:, ds(sti * 128, 128)],
                                wo_sb[:, ds(c * D_MODEL + e2 * NQ, NQ)],
                                start=(c == 0), stop=(c == 1))
                        nc.vector.tensor_copy(ysb[:, ds(e2 * NQ, NQ)], py[:, 0:NQ])
                    eng = nc.sync if sti % 2 == 0 else nc.gpsimd
                    eng.dma_start(
                        out=y[sti * 128:(sti + 1) * 128, :],
                        in_=ysb)

            emit_b(0)
            emit_attn(0, 0)
            emit_b(1)
            emit_attn(1, 0)
            emit_attn(0, 1)
            emit_e(0)
            emit_attn(1, 1)
            emit_attn(0, 2)
            emit_e(1)
            emit_attn(1, 2)
            emit_attn(0, 3)
            emit_e(2)
            emit_attn(1, 3)
            emit_e(3)
    nc.finalize()
    return nc


def _host_prep(x, Wq, Wk, Wv, Wo):
    x = np.asarray(x, dtype=np.float32)
    Wq, Wk, Wv, Wo = (np.asarray(w, dtype=np.float32) for w in (Wq, Wk, Wv, Wo))
    bf = ml_dtypes.bfloat16

    p64 = np.concatenate([np.arange(0, DK, 2), np.arange(1, DK, 2)])
    freqs = 1.0 / THETA ** (np.arange(0, DK, 2, dtype=np.float64) / DK)
    ang = np.arange(S, dtype=np.float64)[None, :] * freqs[:, None]
    cos32 = np.cos(ang).astype(np.float32)
    sin32 = np.sin(ang).astype(np.float32)
    cosT = np.ascontiguousarray(np.tile(cos32, (4, 1))).astype(bf)
    # sin_alt: pre-swap layout [s, -s, s, -s]; the kernel multiplies FIRST,
    # then swaps 32-blocks, landing [-s, s, -s, s] contributions.
    sinT = np.ascontiguousarray(
        np.concatenate([sin32, -sin32, sin32, -sin32], axis=0)).astype(bf)

    xTs = [np.ascontiguousarray(x[b].T).astype(bf) for b in range(B)]
    perm = np.concatenate([h * DK + p64 for h in range(HPC)])

    in_maps = []
    for core in range(8):
        bg, hg = core // 4, core % 4
        sl = slice(hg * EL, (hg + 1) * EL)
        in_maps.append({
            "xT": xTs[bg],
            "wq": np.ascontiguousarray(Wq[sl][perm].T).astype(bf),
            "wk": np.ascontiguousarray(Wk[sl][perm].T).astype(bf),
            "wv": np.ascontiguousarray(Wv[sl].T).astype(bf),
            "wo": np.ascontiguousarray(Wo[:, sl].T).astype(bf),
            "cosT": cosT,
            "sinT": sinT,
        })
    return in_maps


def kernel(x, Wq, Wk, Wv, Wo, _trace=False):
    if "nc" not in _CACHE:
        _CACHE["nc"] = _build_nc()
    nc = _CACHE["nc"]
    in_maps = _host_prep(x, Wq, Wk, Wv, Wo)
    res = run_bass_kernel_spmd(nc, in_maps, core_ids=list(range(8)), trace=_trace)
    _CACHE["last_result"] = res
    out = np.zeros((B, S, D_MODEL), dtype=np.float32)
    for core in range(8):
        out[core // 4] += np.asarray(res.results[core]["y"], dtype=np.float32)
    return out
